# revision 29
# baseline (speedup 1.0000x reference)
"""Trainium2 Bass kernel for ColRepeatCausalLinear:

    decay   = clip(decay_value, 0.9, 1.0)
    cache_t = decay * cache_{t-1} + x_t          (scan along T, per (b, d))
    out_t   = weight[t] * cache_t + bias[t]

Shapes: x [B=8, T=4096, D=1024] f32, weight [1, T], bias [T], decay_value [1].

Strategy (one batch per NeuronCore, 8 cores):
  - Chunk T into 32 blocks of 128.  Within a chunk the scan is a matmul
    with the lower-triangular decay matrix L[t, s] = decay^(t-s) (t >= s):
    cache_k = L @ x_k, computed on the TensorEngine in fp32 with the
    chunk rows on partitions and D on the free axis (no transposes).
  - The cross-chunk carry folds into the next chunk's matmul through
    row 0: since L[t, 0] = decay^t, adding decay*carry to x_{k+1}[0, :]
    makes L @ x' produce the full prefix (carry term decay^{t+1}*carry).
    carry = cache_k[127, :].  Engine APs must start on a 32-aligned
    partition, so the matmul's M ordering is rotated by one (PSUM
    partition 0 holds cache[127], partition m holds cache[m-1]); the
    carry patch (one DVE scalar_tensor_tensor per chunk half) then reads
    PSUM partition 0.
  - D is split into two halves (one PSUM bank each) forming two
    independent carry chains, interleaved so the PE never idles on the
    serial patch latency.
  - out = weight[t]*cache + bias[t] is a ScalarEngine activation
    (Identity) with per-partition scale/bias APs, PSUM -> SBUF, written
    in the rotated row order; a small follow-up ACT overwrites each
    staging slot's partition-0 lane with the previous chunk's final row
    so one affine full-128-partition DMA per staging group stores
    everything (128-partition DMAs spray across all 16 SDMA engines;
    odd partition counts collapse onto one engine at ~26 GB/s).
  - Queue separation: inputs on the Sync HWDGE ring, output stores on
    the GpSimd SWDGE queue, so no DMA completion-wait ever heads a
    queue that something else needs.
"""

import numpy as np

B, T, D = 8, 4096, 1024
CH = 128                 # chunk rows (PE contraction dim)
NK = T // CH             # 32 chunks
CPG = 4                  # chunks per DMA staging group
NG = NK // CPG           # 8 staging groups
NH = 2                   # d-halves (carry chains)
DH = D // NH             # 512 = one PSUM bank of fp32
# ramped staging-group sizes (in chunks); must sum to NK
GROUPS = [1, 1] + [2] * 14 + [1, 1]
assert sum(GROUPS) == NK

_CACHED = {}


def _build_program(decay: float):
    import concourse.mybir as mybir
    from concourse import bacc
    from concourse.tile import TileContext

    f32 = mybir.dt.float32
    nc = bacc.Bacc("TRN2", target_bir_lowering=False,
                   disable_frame_to_traceback=True)

    x_d = nc.dram_tensor("x", [T, D], f32, kind="ExternalInput")
    lt_d = nc.dram_tensor("lt", [CH, CH], f32, kind="ExternalInput")
    w_d = nc.dram_tensor("w", [CH, NK], f32, kind="ExternalInput")
    b_d = nc.dram_tensor("b", [CH, NK], f32, kind="ExternalInput")
    y_d = nc.dram_tensor("y", [T, D], f32, kind="ExternalOutput")

    with TileContext(nc) as tc:
        with (
            tc.tile_pool(name="const", bufs=1) as const,
            tc.tile_pool(name="xin", bufs=2) as xpool,
            tc.tile_pool(name="oout", bufs=6) as opool,
            tc.tile_pool(name="psum", bufs=4, space="PSUM") as pspool,
        ):
            # Ramped staging-group sizes: small first groups so compute
            # starts as soon as possible; small last groups so the tail
            # (ACT + store of the final group) is short.
            lt = const.tile([CH, CH], f32)
            nc.sync.dma_start(out=lt[:], in_=lt_d[:])
            wsb = const.tile([CH, NK], f32)
            bsb = const.tile([CH, NK], f32)

            # Output staging (per group of `cpg` chunks): ot has cpg
            # slots; the ACT for chunk k0+c writes slot c in rotated row
            # order (partition 0 = final row of the chunk, partition p =
            # row p-1).  A small follow-up ACT then overwrites slot c's
            # partition-0 lane with the PREVIOUS chunk's final row (read
            # straight from that chunk's PSUM carry row), which makes the
            # single affine group DMA self-contained: it writes output rows
            # [128*k0-1 .. 128*(k0+cpg)-2] via row = 128*(k0+c) + p - 1,
            # full 128 partitions (required for the descriptor spray across
            # all 16 SDMA engines).  No fixup DMAs, so the SWDGE queue
            # carries only back-to-back group stores and never stalls on a
            # completion wait.
            prev_ps = [None] * NH
            prev_k = None
            k0 = 0
            for grp, cpg in enumerate(GROUPS):
                rows = slice(k0 * CH, (k0 + cpg) * CH)
                xt = xpool.tile([CH, cpg, D], f32, tag=f"xt{cpg}",
                                bufs=4 if cpg == max(GROUPS) else 2)
                nc.sync.dma_start(
                    out=xt[:],
                    in_=x_d[rows, :].rearrange("(c p) d -> p c d", p=CH),
                )
                if grp == 0:
                    # w/b are first needed by the ACT of chunk 0, a few us
                    # after the first matmul; loading them after x keeps
                    # the first compute off the critical startup path
                    nc.sync.dma_start(out=wsb[:], in_=w_d[:])
                    nc.sync.dma_start(out=bsb[:], in_=b_d[:])
                ot = opool.tile([CH, cpg, D], f32, tag=f"ot{cpg}",
                                bufs=4 if cpg == max(GROUPS) else 2)
                for c in range(cpg):
                    k = k0 + c
                    carry_ps = [None] * NH
                    new_ps = [None] * NH
                    # emission order: both patches, then both matmuls
                    # back-to-back (denser PE bursts), then both ACTs
                    for h in range(NH):
                        hs = slice(h * DH, (h + 1) * DH)
                        new_ps[h] = pspool.tile([CH, DH], f32, tag=f"ps{h}", name=f"ps{h}")
                        if k > 0:
                            # x_k[0, :] += decay * cache_{k-1}[127, :]
                            # (carry row sits at PSUM partition 0)
                            nc.vector.scalar_tensor_tensor(
                                out=xt[0:1, c, hs],
                                in0=prev_ps[h][0:1, :],
                                scalar=float(decay),
                                in1=xt[0:1, c, hs],
                                op0=mybir.AluOpType.mult,
                                op1=mybir.AluOpType.add,
                            )
                    for h in range(NH):
                        hs = slice(h * DH, (h + 1) * DH)
                        nc.tensor.matmul(new_ps[h][:], lt[:], xt[:, c, hs],
                                         start=True, stop=True)
                    for h in range(NH):
                        hs = slice(h * DH, (h + 1) * DH)
                        nc.scalar.activation(
                            ot[:, c, hs],
                            new_ps[h][:],
                            mybir.ActivationFunctionType.Identity,
                            bias=bsb[:, k:k + 1],
                            scale=wsb[:, k:k + 1],
                        )
                        carry_ps[h] = prev_ps[h]
                        prev_ps[h] = new_ps[h]
                    if k > 0:
                        # overwrite slot c partition 0 with chunk k-1's
                        # final output row (WAW-ordered after the big ACT)
                        for h in range(NH):
                            nc.scalar.activation(
                                ot[0:1, c, h * DH:(h + 1) * DH],
                                carry_ps[h][0:1, :],
                                mybir.ActivationFunctionType.Identity,
                                bias=bsb[0:1, prev_k:prev_k + 1],
                                scale=wsb[0:1, prev_k:prev_k + 1],
                            )
                    prev_k = k
                r0 = k0 * CH
                if grp > 0:
                    y_win = y_d[r0 - 1:r0 + cpg * CH - 1, :].rearrange(
                        "(c p) d -> p c d", p=CH)
                    nc.gpsimd.dma_start(out=y_win, in_=ot[:])
                else:
                    # group 0: no row -1; chunk 0's body rows 0..126 on
                    # their own, remaining chunks via the affine window
                    nc.gpsimd.dma_start(out=y_d[0:CH - 1, :], in_=ot[1:CH, 0])
                    if cpg > 1:
                        y_win = y_d[CH - 1:cpg * CH - 1, :].rearrange(
                            "(c p) d -> p c d", p=CH)
                        nc.gpsimd.dma_start(out=y_win, in_=ot[:, 1:cpg])
                k0 += cpg
            # final output row T-1 = chunk 31's final row, straight from its
            # PSUM carry row through a last tiny ACT + DMA
            ft = const.tile([1, D], f32)
            for h in range(NH):
                nc.scalar.activation(
                    ft[0:1, h * DH:(h + 1) * DH],
                    prev_ps[h][0:1, :],
                    mybir.ActivationFunctionType.Identity,
                    bias=bsb[0:1, NK - 1:NK],
                    scale=wsb[0:1, NK - 1:NK],
                )
            nc.gpsimd.dma_start(out=y_d[T - 1:T, :], in_=ft[:])
    nc.compile()
    return nc


def _host_constants(weight, bias, decay):
    """L^T with M rotated by one, plus rotated per-chunk w/b columns."""
    t = np.arange(CH)
    diff = t[:, None] - t[None, :]
    L = np.where(diff >= 0, np.float32(decay) ** diff.astype(np.float32), 0.0)
    L = L.astype(np.float32)
    Lrot = np.roll(L, 1, axis=0)        # row m of Lrot = L row (m-1)%128
    LT = np.ascontiguousarray(Lrot.T)   # lhsT[s, m] = L[(m-1)%128, s]
    WT = np.roll(weight.reshape(NK, CH).T.astype(np.float32), 1, axis=0)
    BT = np.roll(bias.reshape(NK, CH).T.astype(np.float32), 1, axis=0)
    return LT, np.ascontiguousarray(WT), np.ascontiguousarray(BT)


def kernel(x, weight, bias, decay_value):
    from concourse.bass_utils import run_bass_kernel_spmd

    x = np.ascontiguousarray(np.asarray(x, dtype=np.float32))
    weight = np.asarray(weight, dtype=np.float32)
    bias = np.asarray(bias, dtype=np.float32)
    decay = float(np.float32(np.clip(np.asarray(decay_value)[0], 0.9, 1.0)))

    LT, WT, BT = _host_constants(weight, bias, decay)

    key = round(decay, 10)
    if key not in _CACHED:
        _CACHED[key] = _build_program(decay)
    nc = _CACHED[key]

    in_maps = [
        {"x": x[b], "lt": LT, "w": WT, "b": BT} for b in range(B)
    ]
    res = run_bass_kernel_spmd(nc, in_maps, core_ids=list(range(B)))
    out = np.stack([res.results[b]["y"] for b in range(B)], axis=0)
    return out.astype(np.float32)
